# revision 5
# baseline (speedup 1.0000x reference)
"""Trainium2 Bass kernel for BCNet-style fused block — fp8 hi/lo split.

Reference computation (per batch b):
    v_ = relu(v @ Wv.T + bv)            # [B, NO, H]
    q_ = relu(q @ Wq.T + bq)            # [B, Q,  H]
    qw = einsum("bqh,q->bh", q_, wh)    # [B, H]
    logits = v_ * qw[:, None, :] + bh   # [B, NO, H]
    out = logits @ W2.T + b2            # [B, NO, VD]

Strategy: pure data parallel over batch (16 per core x 8 cores), weights
replicated. Every matmul operand x is split x = x_hi + x_lo (both fp8 e4m3,
power-of-2 pre-scaling so values sit in the normal range), and each logical
matmul A@B runs as fp8 DoubleRow instructions:
  - main: one instruction per k-tile PAIR computing Ah_k0@Bh_k0 + Ah_k1@Bh_k1
  - corr: one instruction per k-tile computing  Ah_k@Bl_k + Al_k@Bh_k
All accumulate into the same fp32 PSUM group, so per logical matmul the PE
does 1.5 k-passes of DoubleRow work = 0.75x the bf16 cost, with quantization
error ~0.1% per matmul (lo*lo term dropped). The corr instructions for
the top `skip1` k-tiles of MM1 are dropped entirely (spends part of the
2e-2 error budget for PE time; measured rel l2 1.4e-2 at skip1=3).

Scale folding: v,q scaled by 4; Wv,Wq,W2 by 64; logits by 4 (folded into wh
on host). PSUM values are 256x the true values; evictions apply
activation(scale=1/256, bias=...). bh is folded into b2eff on host.

SBUF sub-layouts (s = hi/lo index):
  weights  [P, s(hi,lo), k, cols]   acts  [P, s(lo,hi), k, cols]
so a correction instruction's lhsT = w[:, 0:2, k, m-slice] pairs with
rhs = a[:, 0:2, k, n-slice] to give exactly (Wh@Al + Wl@Ah).

Phases (PE order): warmup -> B matmuls m0-7 (relu-only evictions stashed)
-> A (q-path) + qw -> deferred logit muls for m0-7 -> B m8-15 inline
-> C (out = logits8 @ W2split). DMA is hand-paced on the sync queue.
"""

import os
import sys

import numpy as np

for _p in ("/opt/trn_rl_repo", "/root/.axon_site/_ro/trn_rl_repo"):
    if os.path.isdir(_p) and _p not in sys.path:
        sys.path.insert(0, _p)

import ml_dtypes

import concourse.bacc as bacc
import concourse.bass as bass
import concourse.mybir as mybir
import concourse.tile as tile
from concourse.bass_utils import run_bass_kernel_spmd

B, NO, Q = 128, 36, 14
VD, QD, H = 2048, 1024, 2048
NCORES = 8
BS = B // NCORES          # 16 batches per core
NROW = BS * NO            # 576 v-rows per core
QROW = BS * Q             # 224 q-rows per core
P = 128
NT = 144                  # n-tile (4 batches * 36); DoubleRow rhs free=288<=512
NN = NROW // NT           # 4
BPT = NT // NO            # 4 batches per n-tile
KV = VD // P              # 16 contraction tiles for matmul 1
KQ = QD // P              # 8  contraction tiles for matmul 2
MH = H // P               # 16 output h-tiles
KH = H // P               # 16 contraction tiles for matmul 3
MV = VD // P              # 16 output vd-tiles

F32 = mybir.dt.float32
BF16 = mybir.dt.bfloat16
FP8 = mybir.dt.float8e4
E4_NP = ml_dtypes.float8_e4m3
BF16_NP = ml_dtypes.bfloat16
DR = mybir.MatmulPerfMode.DoubleRow

SV = 4.0     # activation scale (v, q)
SW = 64.0    # weight scale (Wv, Wq, W2)
SL = 4.0     # logits scale (folded into wh on host)
INV = 1.0 / 256.0   # eviction scale: 1/(SV*SW) = 1/(SL*SW)

WV_CB = 512          # Wv/W2 column-block width -> 4 blocks, 16KB tiles
WQ_CB = 1024         # Wq column-block width -> 2 blocks, 16KB tiles


def _build_program(opts=None):
    o = dict(
        warmup=70,
        wv_kchunk=8,      # k-tiles per DMA chunk within a Wv/W2 block
        wv0_ck=8,         # finer chunking for the first Wv block
        tail_split=2,     # sub-splits of the final output group
        kint=16,          # k-tiles per interleaved (main+corr) sub-chunk
        kint0=16,         # window granularity for the first batch + block 0
        front_order="vfirst",  # block-0 stream: vfirst | vinter
        nt_c=144,         # phase-C n-tile width (must divide 576, <=256)
        tail_dve=0,       # evict final C groups on DVE instead of ACT
        last_scalar=1,    # issue the final out-DMA from the ACT queue
        phase_v2=0,       # B(m0-11) -> A -> evicts -> B(m12-15) ordering
        b1_order="nm",    # first-half B group order: n-major or m-major
        b2_order="nm",    # second-half B group order
        out_eng="alt",    # output DMA queues: sync/scalar/alt(sync+vector)
        skip1=3,          # corr k-tiles skipped (from top) in MM1
        skip3=0,          # corr k-tiles skipped (from top) in MM3
        c_windows=0,      # reordered C windows (m0-7 first per n-pair)
    )
    if opts:
        o.update(opts)

    nc = bacc.Bacc("TRN2", target_bir_lowering=False, debug=False,
                   num_devices=NCORES)

    # DRAM tensors (all pre-split/interleaved on host)
    vT = nc.dram_tensor("vT", [P, NN * 2 * KV * NT], FP8,
                        kind="ExternalInput").ap()
    qT = nc.dram_tensor("qT", [P, 2 * KQ * QROW], FP8,
                        kind="ExternalInput").ap()
    WvT = nc.dram_tensor("WvT", [P, 2 * KV * H], FP8,
                         kind="ExternalInput").ap()
    WqT = nc.dram_tensor("WqT", [P, 2 * KQ * H], FP8,
                         kind="ExternalInput").ap()
    W2T = nc.dram_tensor("W2T", [P, 2 * KH * VD], FP8,
                         kind="ExternalInput").ap()
    constC = nc.dram_tensor("constC", [P, 3 * 16 + QROW], F32,
                            kind="ExternalInput").ap()
    outT = nc.dram_tensor("outT", [VD, NROW], F32, kind="ExternalOutput").ap()

    vT_r = vT.rearrange("p (n s k c) -> p n s k c", n=NN, s=2, k=KV)
    qT_r = qT.rearrange("p (s k c) -> p s k c", s=2, k=KQ)
    WvT_r = WvT.rearrange("p (s k c) -> p s k c", s=2, k=KV)
    WqT_r = WqT.rearrange("p (s k c) -> p s k c", s=2, k=KQ)
    W2T_r = W2T.rearrange("p (s k c) -> p s k c", s=2, k=KH)

    NWV = H // WV_CB      # 4
    NWQ = H // WQ_CB      # 2
    NW2 = VD // WV_CB     # 4

    with tile.TileContext(nc) as tc:
        from contextlib import ExitStack

        with ExitStack() as ctx:
            wpool = ctx.enter_context(tc.tile_pool(name="weights", bufs=7))
            apool = ctx.enter_context(tc.tile_pool(name="acts", bufs=1))
            lpool = ctx.enter_context(tc.tile_pool(name="logits", bufs=1))
            qwpool = ctx.enter_context(tc.tile_pool(name="qw", bufs=MH))
            const = ctx.enter_context(tc.tile_pool(name="const", bufs=1))
            stage = ctx.enter_context(tc.tile_pool(name="stage", bufs=6))
            vspool = ctx.enter_context(tc.tile_pool(name="vstash", bufs=56))
            lfpool = ctx.enter_context(tc.tile_pool(name="lf", bufs=14))
            ospool = ctx.enter_context(tc.tile_pool(name="ostage", bufs=10))
            psum = ctx.enter_context(
                tc.tile_pool(name="psum", bufs=8, space="PSUM"))

            # Consts packed into one DMA: bv | bq | b2eff | wh_eff
            cst = const.tile([P, 3 * 16 + QROW], F32)
            bv_sb = cst[:, 0:16]
            bq_sb = cst[:, 16:32]
            b2_sb = cst[:, 32:48]
            wh_sb = cst[:, 48:48 + QROW]

            if o["warmup"]:
                wup = stage.tile([P, 64], BF16, tag="wup", name="wup")
                nc.gpsimd.memset(wup[:], 0.0)
                wps = psum.tile([64, 64], F32, tag="ps", name="pswarm")
                for _ in range(o["warmup"]):
                    nc.tensor.matmul(wps[:], lhsT=wup[:, 0:64], rhs=wup[:],
                                     start=True, stop=True)

            # SBUF tiles
            vt = apool.tile([P, NN, 2, KV, NT], FP8, name="vt")
            qt = apool.tile([P, 2, KQ, QROW], FP8, name="qt")
            lts = lpool.tile([P, 2, KH, NROW], FP8, name="lts")
            wvts = [wpool.tile([P, 2, KV, WV_CB], FP8, tag="w", name=f"wv{s}")
                    for s in range(NWV)]
            wqts = [wpool.tile([P, 2, KQ, WQ_CB], FP8, tag="w", name=f"wq{s}")
                    for s in range(NWQ)]
            w2ts = [wpool.tile([P, 2, KH, WV_CB], FP8, tag="w", name=f"w2{s}")
                    for s in range(NW2)]

            # ---- DMA helpers (sync queue; emission order == transfer order)
            def dma_cst():
                nc.sync.dma_start(out=cst[:], in_=constC)

            def dma_v(n):
                nc.sync.dma_start(out=vt[:, n], in_=vT_r[:, n])

            def dma_vp(n, s, k0, k1):
                nc.sync.dma_start(out=vt[:, n, s, k0:k1, :],
                                  in_=vT_r[:, n, s, k0:k1, :])

            def dma_q():
                nc.sync.dma_start(out=qt[:], in_=qT_r)

            def dma_wv(s, sub, k0, k1):
                nc.sync.dma_start(
                    out=wvts[s][:, sub, k0:k1, :],
                    in_=WvT_r[:, sub, k0:k1, s * WV_CB:(s + 1) * WV_CB])

            def dma_wq(s, sub):
                nc.sync.dma_start(
                    out=wqts[s][:, sub],
                    in_=WqT_r[:, sub, :, s * WQ_CB:(s + 1) * WQ_CB])

            def dma_w2(s, sub):
                nc.sync.dma_start(
                    out=w2ts[s][:, sub],
                    in_=W2T_r[:, sub, :, s * WV_CB:(s + 1) * WV_CB])

            def dma_wv_block(s):
                for (s_, sub, k0, k1) in wv_chunks(s, lo_kmax=KV - o["skip1"]):
                    dma_wv(s_, sub, k0, k1)

            # DMA stream order (hand-paced to PE consumption). Wv blocks are
            # emitted in (sub, k-chunk) pieces interleaved with the v n-tiles
            # so B's k-interleaved groups can start as chunks land.
            kint = o["kint"]

            def wv_chunks(s, lo_kmax=None):
                # (sub, k) pieces in the order B's k-interleaved groups
                # consume them: per kint-range, hi chunks then lo chunks.
                # lo-sub chunks above lo_kmax are never read (skipped corrs).
                ck = o["wv0_ck"] if s == 0 else o["wv_kchunk"]
                lo_kmax = KV if lo_kmax is None else lo_kmax
                for kc in range(0, KV, kint):
                    for sub in range(2):
                        for c in range(kc, kc + kint, ck):
                            c1 = min(c + ck, lo_kmax) if sub == 1 else c + ck
                            if c1 > c:
                                yield (s, sub, c, c1)

            # batch (m0-3, n0-1): per window, v hi pieces + wv hi chunks,
            # then v lo pieces + wv lo chunks (matching win-major PE order).
            # cst rides after the first wv chunk (first needed by B relu).
            ck0 = o["wv0_ck"]
            kint0 = o["kint0"]
            first_chunk = True
            for kc in range(0, KV, kint0):
                for sub_pe, sub_v in ((0, 1), (1, 0)):   # w-hi/v-hi, w-lo/v-lo
                    dma_vp(0, sub_v, kc, kc + kint0)
                    dma_vp(1, sub_v, kc, kc + kint0)
                    for c in range(kc, kc + kint0, ck0):
                        dma_wv(0, sub_pe, c, c + ck0)
                        if first_chunk:
                            dma_cst()
                            first_chunk = False
            # batch (m0-3, n2-3): v n2, n3 pieces in window order
            for kc in range(0, KV, kint):
                for sub_v in (1, 0):
                    dma_vp(2, sub_v, kc, kc + kint)
                    dma_vp(3, sub_v, kc, kc + kint)
            dma_wv_block(1)       # B m4-7
            if o["phase_v2"]:
                dma_wv_block(2)   # B m8-11 comes before the q-path stream
                dma_q()
                dma_wq(0, 0)      # A m0-7 (hi then lo)
                dma_wq(0, 1)
                dma_wq(1, 0)      # A m8-15
                dma_wq(1, 1)
                dma_wv_block(3)   # B m12-15
            else:
                dma_q()
                dma_wq(0, 0)
                dma_wq(0, 1)
                dma_wq(1, 0)
                dma_wq(1, 1)
                dma_wv_block(2)
                dma_wv_block(3)
            for s in range(NW2):  # C
                dma_w2(s, 0)
                dma_w2(s, 1)

            # ---- matmul slice helpers
            def wv_main(k0, m):
                s, r = divmod(m * P, WV_CB)
                return wvts[s][:, 0, k0:k0 + 2, r:r + P]

            def wv_corr(k, m):
                s, r = divmod(m * P, WV_CB)
                return wvts[s][:, 0:2, k, r:r + P]

            def wq_main(k0, m):
                s, r = divmod(m * P, WQ_CB)
                return wqts[s][:, 0, k0:k0 + 2, r:r + P]

            def wq_corr(k, m):
                s, r = divmod(m * P, WQ_CB)
                return wqts[s][:, 0:2, k, r:r + P]

            def w2_main(k0, m):
                s, r = divmod(m * P, WV_CB)
                return w2ts[s][:, 0, k0:k0 + 2, r:r + P]

            def w2_corr(k, m):
                s, r = divmod(m * P, WV_CB)
                return w2ts[s][:, 0:2, k, r:r + P]

            def split_group(ps, wmain, wcorr, rmain, rcorr, nk, ki=None,
                            skip=0):
                """Emit one full hi/lo-split accumulation group into psum ps.

                wmain(k0) / rmain(k0): 2-k-tile hi slices; wcorr(k)/rcorr(k):
                (hi,lo)x(lo,hi) 1-k-tile pair slices. nk = # 128-k-tiles.
                ki: k-interleave granularity (mains then corrs per ki-range),
                matching the (hi,lo)-per-ki-range DMA chunk order.
                skip: drop the corr instructions for the top `skip` k-tiles
                (spends error budget for PE time).
                """
                ki = ki or nk
                klast = nk - 1 - skip
                for kc in range(0, nk, ki):
                    for k0 in range(kc, kc + ki, 2):
                        nc.tensor.matmul(ps[:], lhsT=wmain(k0), rhs=rmain(k0),
                                         start=(k0 == 0), stop=False,
                                         perf_mode=DR)
                    for k in range(kc, kc + ki):
                        if k > klast:
                            continue
                        nc.tensor.matmul(ps[:], lhsT=wcorr(k), rhs=rcorr(k),
                                         start=False, stop=(k == klast),
                                         perf_mode=DR)

            qwts = [None] * MH
            vstash = {}

            def b_relu(m, n, ps):
                vs = vspool.tile([P, NT], F32, tag="vs", name=f"vs{m}_{n}")
                nc.scalar.activation(vs[:], ps[:],
                                     mybir.ActivationFunctionType.Relu,
                                     bias=bv_sb[:, m:m + 1], scale=INV)
                vstash[(m, n)] = vs

            def b_batch(groups, ki=None):
                """Win-major B matmuls across up to 8 (m, n) groups.

                Per kint-window: mains of every group, then corrs of every
                group — matching the DMA chunk order, so PE never blocks on
                one group's next window while another group's data is ready.
                """
                ki = ki or kint
                klast = KV - 1 - o["skip1"]
                pss = {g: psum.tile([P, NT], F32, tag="ps",
                                    name=f"psB{g[0]}_{g[1]}")
                       for g in groups}
                for kc in range(0, KV, ki):
                    for (m, n) in groups:
                        for k0 in range(kc, kc + ki, 2):
                            nc.tensor.matmul(
                                pss[(m, n)][:], lhsT=wv_main(k0, m),
                                rhs=vt[:, n, 1, k0:k0 + 2, :],
                                start=(k0 == 0), stop=False, perf_mode=DR)
                    for (m, n) in groups:
                        for k in range(kc, kc + ki):
                            if k > klast:
                                continue
                            nc.tensor.matmul(
                                pss[(m, n)][:], lhsT=wv_corr(k, m),
                                rhs=vt[:, n, 0:2, k, :],
                                start=False, stop=(k == klast),
                                perf_mode=DR)
                return pss

            def b_group(m, n):
                """Phase-B matmuls for tile (m, n) + relu eviction to stash."""
                ps = psum.tile([P, NT], F32, tag="ps", name=f"psB{m}_{n}")
                split_group(
                    ps,
                    lambda k0: wv_main(k0, m), lambda k: wv_corr(k, m),
                    lambda k0: vt[:, n, 1, k0:k0 + 2, :],
                    lambda k: vt[:, n, 0:2, k, :],
                    KV, ki=kint, skip=o["skip1"])
                b_relu(m, n, ps)

            def evict_window(pairs):
                """Batched logit production for a set of (m, n) tiles:
                all DVE muls, then all ACT casts, then all DVE subs — no
                per-chain cross-engine round-trip stalls on in-order DVE."""
                lfs = {}
                for (m, n) in pairs:
                    vs = vstash.pop((m, n))
                    lf = lfpool.tile([P, NT], F32, tag="lf",
                                     name=f"lf{m}_{n}")
                    qb = qwts[m][:, n * BPT:(n + 1) * BPT].to_broadcast(
                        [P, BPT, NO])
                    nc.vector.tensor_mul(
                        lf.rearrange("p (b o) -> p b o", b=BPT),
                        vs.rearrange("p (b o) -> p b o", b=BPT), qb)
                    lfs[(m, n)] = lf
                for (m, n) in pairs:
                    nsl = slice(n * NT, (n + 1) * NT)
                    nc.gpsimd.tensor_copy(lts[:, 1, m, nsl], lfs[(m, n)][:])
                for (m, n) in pairs:
                    nsl = slice(n * NT, (n + 1) * NT)
                    if m < KH - o["skip3"]:
                        nc.vector.tensor_sub(lts[:, 0, m, nsl],
                                             lfs[(m, n)][:],
                                             lts[:, 1, m, nsl])

            def b_evict(m, n):
                """Deferred logit production: lf = vs*qb; lh, ll -> lts."""
                vs = vstash.pop((m, n))
                lf = lfpool.tile([P, NT], F32, tag="lf", name=f"lf{m}_{n}")
                qb = qwts[m][:, n * BPT:(n + 1) * BPT].to_broadcast(
                    [P, BPT, NO])
                nc.vector.tensor_mul(
                    lf.rearrange("p (b o) -> p b o", b=BPT),
                    vs.rearrange("p (b o) -> p b o", b=BPT), qb)
                nsl = slice(n * NT, (n + 1) * NT)
                nc.gpsimd.tensor_copy(lts[:, 1, m, nsl], lf[:])
                if m < KH - o["skip3"]:
                    nc.vector.tensor_sub(lts[:, 0, m, nsl], lf[:],
                                         lts[:, 1, m, nsl])

            def a_group(m):
                ps = psum.tile([P, QROW], F32, tag="ps", name=f"psA{m}")
                split_group(
                    ps,
                    lambda k0: wq_main(k0, m), lambda k: wq_corr(k, m),
                    lambda k0: qt[:, 1, k0:k0 + 2, :],
                    lambda k: qt[:, 0:2, k, :],
                    KQ)
                qs = stage.tile([P, QROW], F32, tag="qstage", name=f"qs{m}")
                nc.scalar.activation(qs[:], ps[:],
                                     mybir.ActivationFunctionType.Relu,
                                     bias=bq_sb[:, m:m + 1], scale=INV)
                qp = stage.tile([P, QROW], F32, tag="qstage", name=f"qp{m}")
                nc.vector.tensor_mul(qp[:], qs[:], wh_sb)
                qw = qwpool.tile([P, BS], F32, tag="qw", name=f"qw{m}")
                nc.vector.tensor_reduce(
                    qw[:], qp.rearrange("p (b q) -> p b q", b=BS),
                    axis=mybir.AxisListType.X, op=mybir.AluOpType.add)
                qwts[m] = qw

            def group_order(ms, mode):
                if mode == "nm":
                    return [(m, n) for n in range(NN) for m in ms]
                return [(m, n) for m in ms for n in range(NN)]

            if o["phase_v2"]:
                # ---- B m0-11 (matmuls + relu only), then A, then all
                # deferred logit chains n-major, then B m12-15. Keeps the
                # q/wq stream off the critical path to B(m8-11)'s weights.
                first_batch = True
                for ms, ns in (((0, 1, 2, 3), (0, 1)), ((0, 1, 2, 3), (2, 3)),
                               ((4, 5, 6, 7), (0, 1)), ((4, 5, 6, 7), (2, 3)),
                               ((8, 9, 10, 11), (0, 1)),
                               ((8, 9, 10, 11), (2, 3))):
                    pss = b_batch([(m, n) for n in ns for m in ms],
                                  ki=o["kint0"] if first_batch else None)
                    first_batch = False
                    for (m, n), ps in pss.items():
                        b_relu(m, n, ps)
                for m in range(MH):
                    a_group(m)
                for n in range(NN):
                    for mw in (range(0, 6), range(6, 12)):
                        evict_window([(m, n) for m in mw])
                    if n % 2 == 1:
                        # B m12-15 for this n-pair: per-group emission so the
                        # relus spread out (no ACT backlog before C).
                        for nn in (n - 1, n):
                            for m in (12, 13, 14, 15):
                                b_group(m, nn)
                            evict_window([(m, nn)
                                          for m in (12, 13, 14, 15)])
            else:
                # ---- Phase B first half (m0-7): win-major batches, relus
                first_batch = True
                for ms, ns in (((0, 1, 2, 3), (0, 1)), ((0, 1, 2, 3), (2, 3)),
                               ((4, 5, 6, 7), (0, 1)), ((4, 5, 6, 7), (2, 3))):
                    pss = b_batch([(m, n) for n in ns for m in ms],
                                  ki=o["kint0"] if first_batch else None)
                    first_batch = False
                    for (m, n), ps in pss.items():
                        b_relu(m, n, ps)
                # ---- Phase A
                for m in range(MH):
                    a_group(m)
                # ---- deferred logit evictions for m0-7
                for m in range(8):
                    for n in range(NN):
                        b_evict(m, n)
                # ---- Phase B second half (m8-15): inline evictions
                for m, n in group_order(range(8, MH), o["b2_order"]):
                    b_group(m, n)
                    b_evict(m, n)

            # ---- Phase C: outT[vd, :] = (lts_hi+lts_lo) @ W2split + b2eff
            # n-major so the n3 groups (whose logits evict last) come with
            # maximal slack; one output DMA per (m, n) piece.
            out_engs = {"sync": [nc.sync], "scalar": [nc.scalar],
                        "alt": [nc.sync, nc.gpsimd],
                        "alt3": [nc.sync, nc.gpsimd, nc.scalar]}[o["out_eng"]]
            NTC = o["nt_c"]
            NNC = NROW // NTC
            if o["c_windows"]:
                # m8-15 need the late w2 blocks; give the stream two windows
                # of m0-7 head start before touching them.
                c_windows = [(0, range(8)), (1, range(8)),
                             (0, range(8, MV)), (1, range(8, MV)),
                             (2, range(MV)), (3, range(MV))]
            else:
                c_windows = [(n, range(MV)) for n in range(NNC)]
            for wi, (n, mrange) in enumerate(c_windows):
                for m in mrange:
                    last = (wi == len(c_windows) - 1 and m == MV - 1)
                    nsub = o["tail_split"] if last else 1
                    w = NTC // nsub
                    for h in range(nsub):
                        c0 = n * NTC + h * w
                        hsl = slice(c0, c0 + w)
                        ps = psum.tile([P, w], F32, tag="ps",
                                       name=f"psC{m}_{n}_{h}")
                        split_group(
                            ps,
                            lambda k0: w2_main(k0, m),
                            lambda k: w2_corr(k, m),
                            lambda k0: lts[:, 1, k0:k0 + 2, hsl],
                            lambda k: lts[:, 0:2, k, hsl],
                            KH, skip=o["skip3"])
                        os_ = ospool.tile([P, w], F32, tag="os",
                                          name=f"os{m}_{n}_{h}")
                        if last and o["tail_dve"]:
                            nc.vector.scalar_tensor_tensor(
                                os_[:], in0=ps[:], scalar=INV,
                                op0=mybir.AluOpType.mult,
                                op1=mybir.AluOpType.add,
                                in1=b2_sb[:, m:m + 1].to_broadcast([P, w]))
                        else:
                            nc.scalar.activation(
                                os_[:], ps[:],
                                mybir.ActivationFunctionType.Identity,
                                bias=b2_sb[:, m:m + 1], scale=INV)
                        if last and h == nsub - 1 and o["last_scalar"]:
                            eng = nc.scalar
                        else:
                            eng = out_engs[(n * MV + m + h) % len(out_engs)]
                        eng.dma_start(
                            out=outT[m * P:(m + 1) * P, hsl], in_=os_[:])

    nc.compile()
    return nc


_NC_CACHE = {}


def get_program(opts=None):
    key = tuple(sorted(opts.items())) if opts else ()
    if key not in _NC_CACHE:
        _NC_CACHE[key] = _build_program(opts)
    return _NC_CACHE[key]


def _split8(x):
    """x (f32) -> (hi, lo) fp8 e4m3 with x ~= hi + lo."""
    hi = x.astype(E4_NP)
    lo = (x - hi.astype(np.float32)).astype(E4_NP)
    return hi, lo


def _prep_weight(W, scale, kt):
    """W.T scaled+split -> [P, 2(hi,lo), kt, cols] flattened per partition."""
    WT = np.ascontiguousarray(W.astype(np.float32).T) * scale  # [K, M]
    K, M = WT.shape
    hi, lo = _split8(WT)
    arr = np.stack([hi, lo])                 # [2, K, M]
    arr = arr.reshape(2, kt, P, M).transpose(2, 0, 1, 3)   # [P, 2, kt, M]
    return np.ascontiguousarray(arr.reshape(P, 2 * kt * M))


def make_in_maps(v, q, Wv, bv, Wq, bq, wh, bh, W2, b2):
    """Host-side prep: shard batch, scale, split to fp8 hi/lo, interleave."""
    WvT8 = _prep_weight(Wv, SW, KV)          # [P, 2*16*2048]
    WqT8 = _prep_weight(Wq, SW, KQ)          # [P, 2*8*2048]
    W2T8 = _prep_weight(W2, SW, KH)          # [P, 2*16*2048]
    b2eff = (b2.astype(np.float64)
             + float(bh) * W2.astype(np.float64).sum(axis=1)).astype(np.float32)
    constC = np.zeros((P, 3 * 16 + QROW), np.float32)
    constC[:, 0:16] = bv.astype(np.float32).reshape(MH, P).T
    constC[:, 16:32] = bq.astype(np.float32).reshape(MH, P).T
    constC[:, 32:48] = b2eff.reshape(MV, P).T
    constC[:, 48:] = np.tile(wh.astype(np.float32) * SL, BS)[None, :]

    in_maps = []
    for c in range(NCORES):
        b0 = c * BS
        v_sh = v[b0:b0 + BS].reshape(NROW, VD).astype(np.float32) * SV
        q_sh = q[b0:b0 + BS].reshape(QROW, QD).astype(np.float32) * SV
        # vT: [P, n, s(lo,hi), k, NT]
        vhi, vlo = _split8(np.ascontiguousarray(v_sh.T))     # [VD, NROW]
        va = np.stack([vlo, vhi])                            # [2, VD, NROW]
        va = (va.reshape(2, KV, P, NN, NT)
              .transpose(2, 3, 0, 1, 4))                     # [P, n, 2, k, NT]
        qhi, qlo = _split8(np.ascontiguousarray(q_sh.T))     # [QD, QROW]
        qa = np.stack([qlo, qhi])                            # [2, QD, QROW]
        qa = (qa.reshape(2, KQ, P, QROW)
              .transpose(2, 0, 1, 3))                        # [P, 2, k, QROW]
        in_maps.append({
            "vT": np.ascontiguousarray(va.reshape(P, NN * 2 * KV * NT)),
            "qT": np.ascontiguousarray(qa.reshape(P, 2 * KQ * QROW)),
            "WvT": WvT8, "WqT": WqT8, "W2T": W2T8,
            "constC": constC,
        })
    return in_maps


def assemble_output(results):
    outs = []
    for c in range(NCORES):
        outT = results[c]["outT"]                      # [VD, NROW] f32
        outs.append(np.ascontiguousarray(outT.T).reshape(BS, NO, VD))
    return np.concatenate(outs, axis=0)


def kernel(v, q, Wv, bv, Wq, bq, wh, bh, W2, b2, **_unused):
    v, q, Wv, bv, Wq, bq, wh, bh, W2, b2 = (
        np.asarray(x) for x in (v, q, Wv, bv, Wq, bq, wh, bh, W2, b2))
    nc = get_program()
    in_maps = make_in_maps(v, q, Wv, bv, Wq, bq, wh, bh, W2, b2)
    res = run_bass_kernel_spmd(nc, in_maps, list(range(NCORES)))
    return assemble_output(res.results)
